# revision 1
# baseline (speedup 1.0000x reference)
"""Trainium2 Bass kernel for nn_EnhancedDLinear (8-core SPMD, full I/O).

Mathematical reductions (verified against the jax reference, exact in fp32):

1. ``LayerNorm(1)`` in the reference normalizes over a size-1 axis, so
   ``(v - mean(v)) == 0`` exactly and its output is the constant ``ln_b``.
   Everything feeding it (detail decomposition, conv stack, adaptive
   softmax, the [N,S,S] self-attention) is dead code; ``detail_pred`` is a
   weight-only constant row, folded on the host.
2. The replicate-pad moving average (k=25) is a linear map ``mt = xc@Mm``;
   ``Mm`` folds into the first trend/seasonal MLP layers.
3. The channel-mean feeding the fusion MLP folds into its weights (1/96)
   and the constant detail contribution into its bias.

Sharding: the folded batch*channel axis (N = B*C) splits into 8 contiguous
blocks of C=96 = exactly one batch per core; each core runs one batch end
to end, zero collectives; tiny weights replicated.

Kernel structure (per core):
- Large matmuls (layer 1/2, softmax layer 2) run as float32r with moving
  dim >= 256 (1 cycle/row); data pre-rounded on host to the fp32r grid
  (sign+8exp+11mant). f32-consumed constants ship in separate f32 tensors
  (a DMA into an f32r tile rounds the payload).
- Layer 2 emits [tp | sp | rowsum(tp) | rowsum(sp)] in one PSUM tile via a
  block-diagonal weight matrix with appended column-sum columns.
- Per-partition biases ride the Relu activations; softmax-layer bias rides
  an augmented K=33 contraction (constant 1 in partition 32); rank-1
  output biases ride broadcast-DMAs + DVE adds.
- One row-contiguous DMA per tensor (DMA cost is per partition-row
  packet), spread across the three DMA-capable queues in need-order.
"""

import numpy as np

import concourse.bacc as bacc
import concourse.tile as tile
from concourse import mybir
from concourse.bass_utils import run_bass_kernel_spmd

B, S, C, P = 8, 336, 96, 96
HID = 168
MAIN_K = 25
N_CORES = 8
KC = 112          # K chunk (336 = 3*112)
NB = 256          # fp32r moving-dim block

M_TILES = [(0, 112), (112, 112), (224, 112)]

# wa [112, 1008] f32r: w1 K-chunks 0-2 (336 cols each)
# wb [112, 768]  f32r: w2 block-diag K-chunks (256 cols each)
WB_LEN = 768
# cf [112, 120] f32: colt (cols 0-2 b1 chunks, 4 b1f, 5 fp1b) | fnp
#    (fn1wT_t/96 | fn1wT_s/96 | fp1wT at cols 8:120)
# small [48, 1280] f32r rows/weights for f32r matmuls:
SM_FN2 = 0       # fn2wT rows 0:32 + fn2b row 32
SM_ONE = 768     # 1.0
SM_LEN = 1280
# sf [48, 608] f32: fp2wT (0:96) | [lt2b|ls2b|dp_row] row (96:384) |
#    fp2b row (384:480)
_CACHE = {}


def _round_fp32r(a):
    # fp32r keeps sign + 8 exp + 11 mantissa bits (low 12 bits zero);
    # round-to-nearest-even on the host so DMA'd bits are pre-rounded.
    u = np.ascontiguousarray(a, np.float32).view(np.uint32)
    low = u & np.uint32(0xFFF)
    base = u & ~np.uint32(0xFFF)
    up = (low > 0x800) | ((low == 0x800) & (((base >> 12) & 1) == 1))
    return (base + (up.astype(np.uint32) << 12)).view(np.float32)


def _mavg_matrix(s, k):
    # mt = xc @ Mm for the replicate-padded moving average
    p = (k - 1) // 2
    m = np.zeros((s, s), np.float64)
    for j in range(s):
        for d in range(-p, p + 1):
            i = min(max(j + d, 0), s - 1)
            m[i, j] += 1.0 / k
    return m.astype(np.float32)


def _build_module():
    f32 = mybir.dt.float32
    f32r = mybir.dt.float32r
    nc = bacc.Bacc("TRN2", target_bir_lowering=False, debug=False,
                   num_devices=N_CORES)

    xb = nc.dram_tensor("xb", [KC, 3 * NB], f32r, kind="ExternalInput")
    wa = nc.dram_tensor("wa", [KC, 3 * S], f32r, kind="ExternalInput")
    wb = nc.dram_tensor("wb", [KC, WB_LEN], f32r, kind="ExternalInput")
    small = nc.dram_tensor("small", [48, SM_LEN], f32r, kind="ExternalInput")
    cf = nc.dram_tensor("cf", [KC, 728], f32, kind="ExternalInput")
    y = nc.dram_tensor("y", [P, P], f32, kind="ExternalOutput")

    AF = mybir.ActivationFunctionType

    with tile.TileContext(nc) as tc:
        with (
            tc.tile_pool(name="wp", bufs=1) as wp,
            tc.tile_pool(name="hp", bufs=1) as hp,
            tc.tile_pool(name="pp", bufs=7, space="PSUM") as pp,
        ):
            xbs = wp.tile([KC, 3 * NB], f32r, tag="xbs")
            was = wp.tile([KC, 3 * S], f32r, tag="was")
            wbs = wp.tile([KC, WB_LEN], f32r, tag="wbs")
            small_s = wp.tile([48, SM_LEN], f32r, tag="small")
            cf_s = wp.tile([KC, 728], f32, tag="cf")

            nc.gpsimd.dma_start(out=was, in_=wa[:, :])
            nc.scalar.dma_start(out=xbs, in_=xb[:, :])
            nc.scalar.dma_start(out=cf_s, in_=cf[:, :])
            nc.sync.dma_start(out=wbs, in_=wb[:, :])
            nc.scalar.dma_start(out=small_s, in_=small[:, :])

            colt = cf_s[:, 0:8]
            fnp = cf_s[0:96, 8:120]
            fn2_s33 = small_s[0:33, SM_FN2:SM_FN2 + 288]
            one_r = small_s[0:1, SM_ONE:SM_ONE + 1]
            fp2_s = cf_s[0:48, 120:216]
            r3row = cf[0:1, 216:504]

            # [lt2b | ls2b | dp_row] and fp2b broadcast over 96 partitions
            r3b = hp.tile([96, 288], f32, tag="r3b")
            nc.gpsimd.dma_start(out=r3b, in_=r3row.broadcast_to((96, 288)))
            fp2bb = hp.tile([96, 96], f32, tag="fp2bb")
            nc.gpsimd.dma_start(out=fp2bb,
                                in_=cf[0:1, 504:600].broadcast_to((96, 96)))

            # ---- layer 1: h1T[u, c] = relu(W1.T @ xc_b.T + b1) ----
            h1c = [hp.tile([KC, 96], f32r, tag=f"h1c_{j}", name=f"h1c_{j}")
                   for j in range(3)]
            for i, (u0, us) in enumerate(M_TILES):
                ps = pp.tile([us, NB], f32, tag="ps")
                for j in range(3):
                    nc.tensor.matmul(
                        ps, was[:, S * j + u0:S * j + u0 + us],
                        xbs[:, NB * j:NB * (j + 1)],
                        start=(j == 0), stop=(j == 2))
                nc.scalar.activation(h1c[i], ps[:, 0:96], AF.Relu,
                                     bias=colt[0:us, i:i + 1])

            # ---- layer 2: [tp | sp | tps | sps] in one psum ----
            ps_l2 = pp.tile([96, NB], f32, tag="ps")
            for j in range(3):
                nc.tensor.matmul(ps_l2, h1c[j],
                                 wbs[:, NB * j:NB * (j + 1)],
                                 start=(j == 0), stop=(j == 2))

            ts2 = hp.tile([96, 2], f32, tag="ts2")
            nc.scalar.activation(ts2, ps_l2[:, 192:194], AF.Copy)
            # biased trend/seasonal blocks (off the softmax chain)
            at = hp.tile([96, 96], f32, tag="at")
            nc.vector.tensor_add(at, ps_l2[:, 0:96], r3b[:, 0:96])
            asl = hp.tile([96, 96], f32, tag="asl")
            nc.vector.tensor_add(asl, ps_l2[:, 96:192], r3b[:, 96:192])

            # ---- fusion softmax over 288 (row layout) ----
            ps_z1 = pp.tile([32, 1], f32, tag="ps")
            nc.tensor.matmul(ps_z1, fnp[:, 0:32], ts2[:, 0:1],
                             start=True, stop=False)
            nc.tensor.matmul(ps_z1, fnp[:, 32:64], ts2[:, 1:2],
                             start=False, stop=True)
            z1s = hp.tile([33, 1], f32r, tag="z1s")
            nc.sync.dma_start(out=z1s[32:33, 0:1],
                              in_=small[0:1, SM_ONE:SM_ONE + 1])
            nc.scalar.activation(z1s[0:32, 0:1], ps_z1, AF.Relu,
                                 bias=colt[0:32, 4:5])

            ps_z2 = pp.tile([1, 288], f32, tag="ps")
            nc.tensor.matmul(ps_z2, z1s, fn2_s33, start=True, stop=True)
            e_row = hp.tile([1, 288], f32, tag="e_row")
            den = hp.tile([1, 1], f32, tag="den")
            nc.scalar.activation(e_row, ps_z2, AF.Exp, accum_out=den)
            recip = hp.tile([1, 1], f32, tag="recip")
            nc.vector.reciprocal(recip, den)
            fw_row = hp.tile([1, 288], f32, tag="fw_row")
            nc.vector.tensor_scalar_mul(fw_row, e_row, recip[0:1, 0:1])

            # fw chunks -> per-partition columns via PE transpose
            fwcols = hp.tile([96, 3], f32, tag="fwcols")
            onef = one_r.bitcast(f32)
            ps_fw = pp.tile([96, 3], f32, tag="ps", name="ps_fw")
            for k in range(3):
                nc.tensor.matmul(ps_fw[:, k:k + 1],
                                 fw_row[0:1, 96 * k:96 * (k + 1)], onef,
                                 is_transpose=True, skip_group_check=True)
            nc.vector.tensor_copy(fwcols, ps_fw)

            # ---- G[c, p] = sum_k fw_k[c] * component_k[c, p] ----
            gt = hp.tile([96, 96], f32, tag="gt")
            nc.vector.tensor_scalar_mul(gt, at, fwcols[:, 0:1])
            gs = hp.tile([96, 96], f32, tag="gs")
            nc.vector.tensor_scalar_mul(gs, asl, fwcols[:, 1:2])
            gd = hp.tile([96, 96], f32, tag="gd")
            nc.vector.tensor_scalar_mul(gd, r3b[:, 192:288], fwcols[:, 2:3])
            ga = hp.tile([96, 96], f32, tag="ga")
            nc.vector.tensor_add(ga, gt, gs)
            g = hp.tile([96, 96], f32, tag="g")
            nc.vector.tensor_add(g, ga, gd)

            # ---- final projection (full fp32) ----
            ps_h = pp.tile([48, 96], f32, tag="ps")
            nc.tensor.matmul(ps_h, fnp[:, 64:112], g, start=True, stop=True)
            hs = hp.tile([48, 96], f32, tag="hs")
            nc.scalar.activation(hs, ps_h, AF.Relu, bias=colt[0:48, 5:6])

            ps_o = pp.tile([96, 96], f32, tag="ps")
            nc.tensor.matmul(ps_o, hs, fp2_s, start=True, stop=True)
            out_s = hp.tile([96, 96], f32, tag="out")
            nc.vector.tensor_add(out_s, ps_o, fp2bb)
            nc.sync.dma_start(out=y[:, :], in_=out_s)

    nc.compile()
    return nc


def _prep_weights(i):
    f = np.float32
    mm = _mavg_matrix(S, MAIN_K)
    w1 = np.empty((S, 2 * HID), f)
    w1[:, :HID] = mm @ i['lt1w'].T.astype(f)
    w1[:, HID:] = (np.eye(S, dtype=f) - mm) @ i['ls1w'].T.astype(f)
    wa = np.empty((KC, 3 * S), f)
    for j in range(3):
        wa[:, S * j:S * (j + 1)] = _round_fp32r(w1[KC * j:KC * (j + 1), :])

    # constant detail_pred row (LayerNorm(1) output == ln_b exactly)
    xf = np.full((S,), f(i['ln_b'][0]), f)
    dp_row = (np.maximum(xf @ i['op1w'].T + i['op1b'], 0)
              @ i['op2w'].T + i['op2b']).astype(f)
    dpm = dp_row.mean(dtype=np.float32)
    b1f = (i['fn1b']
           + dpm * i['fn1w'][:, 2 * C:].sum(1)
           + i['lt2b'].mean(dtype=np.float32) * i['fn1w'][:, 0:C].sum(1)
           + i['ls2b'].mean(dtype=np.float32) * i['fn1w'][:, C:2 * C].sum(1)
           ).astype(f)

    lt2wt = np.ascontiguousarray(i['lt2w'].T, f)
    ls2wt = np.ascontiguousarray(i['ls2w'].T, f)
    w2blk = np.zeros((S, NB), f)
    w2blk[0:HID, 0:96] = lt2wt
    w2blk[HID:, 96:192] = ls2wt
    w2blk[0:HID, 192] = lt2wt.sum(1)
    w2blk[HID:, 193] = ls2wt.sum(1)
    wb = np.zeros((KC, WB_LEN), f)
    for j in range(3):
        wb[:, NB * j:NB * (j + 1)] = _round_fp32r(w2blk[KC * j:KC * (j + 1)])

    b1 = np.concatenate([i['lt1b'], i['ls1b']]).astype(f)
    cf = np.zeros((KC, 728), f)
    for idx, (u0, us) in enumerate(M_TILES):
        cf[0:us, idx] = b1[u0:u0 + us]
    cf[0:32, 4] = b1f
    cf[0:48, 5] = i['fp1b']
    cf[0:96, 8:40] = i['fn1w'][:, 0:C].T / C
    cf[0:96, 40:72] = i['fn1w'][:, C:2 * C].T / C
    cf[0:96, 72:120] = i['fp1w'].T.astype(f)

    small = np.zeros((48, SM_LEN), f)
    small[0:32, SM_FN2:SM_FN2 + 288] = _round_fp32r(
        np.ascontiguousarray(i['fn2w'].T, f))
    small[32, SM_FN2:SM_FN2 + 288] = _round_fp32r(i['fn2b'].astype(f))
    small[0, SM_ONE] = 1.0

    cf[0:48, 120:216] = i['fp2w'].T.astype(f)
    cf[0, 216:312] = i['lt2b']
    cf[0, 312:408] = i['ls2b']
    cf[0, 408:504] = dp_row
    cf[0, 504:600] = i['fp2b']

    return dict(wa=wa, wb=wb, small=small, cf=cf)


def make_in_maps(inputs):
    shared = _prep_weights(inputs)
    x = np.asarray(inputs['x'], np.float32)
    in_maps = []
    for b in range(N_CORES):
        xbp = np.zeros((KC, 3 * NB), np.float32)
        for j in range(3):
            xbp[:, NB * j:NB * j + C] = x[b, KC * j:KC * (j + 1), :]
        in_maps.append(dict(shared, xb=_round_fp32r(xbp)))
    return in_maps


def kernel(**inputs):
    if "nc" not in _CACHE:
        _CACHE["nc"] = _build_module()
    res = run_bass_kernel_spmd(_CACHE["nc"], make_in_maps(inputs),
                               core_ids=list(range(N_CORES)))
    return np.stack([res.results[b]["y"] for b in range(N_CORES)], 0)



# revision 5
# speedup vs baseline: 1.0245x; 1.0245x over previous
"""Trainium2 Bass kernel for nn_EnhancedDLinear (8-core SPMD, full I/O).

Mathematical reductions (vs the jax reference, verified numerically):

1. ``LayerNorm(1)`` output is the constant ``ln_b`` (size-1 normalization
   axis), so the detail branch (conv stack, adaptive softmax, [N,S,S]
   attention) is dead code; ``detail_pred`` is a weight-only constant
   row folded on the host.
2. The replicate-pad moving average (k=25) is a linear map folded into
   the first trend/seasonal MLP layers.
3. The channel-mean feeding the fusion MLP folds into its weights; the
   constant detail contribution folds into its bias.
4. Biases ride the matmuls via constant-one contraction rows (no
   broadcast-DMA bias tiles, no vector adds).
5. The fusion softmax normalizer folds into the final hidden Relu's
   per-partition ``scale`` operand, so unnormalized exponentials flow
   through the combine matmuls.

All matmul operands are bf16 (tolerance is 2e-2; measured ~2.6e-3),
halving DMA bytes vs fp32 and running the PE at 1 cycle/row at any
moving width. PSUM accumulation stays fp32.

Sharding: one batch per core (N = B*C, contiguous blocks of C=96), zero
collectives, tiny weights replicated.
"""

import numpy as np
import ml_dtypes

import concourse.bacc as bacc
import concourse.tile as tile
from concourse import mybir
from concourse.bass_utils import run_bass_kernel_spmd

B, S, C, P = 8, 336, 96, 96
HID = 168
MAIN_K = 25
N_CORES = 8
KC = 112          # contraction chunk (336 = 3*112)

_CACHE = {}


def _mavg_matrix(s, k):
    # mt = xc @ Mm for the replicate-padded moving average
    p = (k - 1) // 2
    m = np.zeros((s, s), np.float64)
    for j in range(s):
        for d in range(-p, p + 1):
            i = min(max(j + d, 0), s - 1)
            m[i, j] += 1.0 / k
    return m.astype(np.float32)


def _bf(a):
    return np.ascontiguousarray(a, np.float32).astype(ml_dtypes.bfloat16)


def _build_module():
    f32 = mybir.dt.float32
    bf16 = mybir.dt.bfloat16
    nc = bacc.Bacc("TRN2", target_bir_lowering=False, debug=False,
                   num_devices=N_CORES)

    xb = nc.dram_tensor("xb", [KC, 3 * C], bf16, kind="ExternalInput")
    wa = nc.dram_tensor("wa", [KC, 3 * S], bf16, kind="ExternalInput")
    wb = nc.dram_tensor("wb", [KC + 1, 3 * 194], bf16, kind="ExternalInput")
    # sp16a [96, 112]: fn1t (0:32) | fn1s (32:64) | fp1wT (64:112)
    sp16a = nc.dram_tensor("sp16a", [96, 112], bf16, kind="ExternalInput")
    # sp16b [49, 480]: fp2aug rows 0:49 (0:96) | Wk rows 0:33 (96+96k) |
    #                  dp_row row 0 (384:480)
    sp16b = nc.dram_tensor("sp16b", [49, 480], bf16, kind="ExternalInput")
    # spf f32 [112, 8]: b1 u-chunks (cols 0:3) | b1f rows 0:32 (col 3) |
    #                   fp1b rows 0:48 (col 4)
    spf = nc.dram_tensor("spf", [KC, 8], f32, kind="ExternalInput")
    y = nc.dram_tensor("y", [P, P], f32, kind="ExternalOutput")

    AF = mybir.ActivationFunctionType

    with tile.TileContext(nc) as tc:
        with (
            tc.tile_pool(name="wp", bufs=1) as wp,
            tc.tile_pool(name="hp", bufs=1) as hp,
            tc.tile_pool(name="pp", bufs=7, space="PSUM") as pp,
        ):
            xbs = wp.tile([KC, 3 * C], bf16, tag="xbs")
            was = wp.tile([KC, 3 * S], bf16, tag="was")
            wbs = wp.tile([KC + 1, 3 * 194], bf16, tag="wbs")
            sp16a_s = wp.tile([96, 112], bf16, tag="sp16a")
            sp16b_s = wp.tile([49, 480], bf16, tag="sp16b")
            spf_s = wp.tile([KC, 8], f32, tag="spf")
            dpb = wp.tile([96, 96], bf16, tag="dpb")

            # DMA issue: wa is the L1 long pole -> its own queue (scalar).
            nc.scalar.dma_start(out=was, in_=wa[:, :])
            nc.sync.dma_start(out=xbs, in_=xb[:, :])
            nc.sync.dma_start(out=wbs, in_=wb[:, :])
            nc.gpsimd.dma_start(out=sp16a_s, in_=sp16a[:, :])
            nc.gpsimd.dma_start(out=sp16b_s, in_=sp16b[:, :])
            nc.sync.dma_start(out=spf_s, in_=spf[:, :])
            nc.gpsimd.dma_start(out=dpb,
                                in_=sp16b[0:1, 384:480].broadcast_to((96, 96)))

            # constant-one rows / tiles (off the DMA critical path)
            h1c = [hp.tile([KC + 1, 96], bf16, tag=f"h1c_{j}",
                           name=f"h1c_{j}") for j in range(3)]
            z1s = hp.tile([33, 1], bf16, tag="z1s")
            hs = hp.tile([49, 96], bf16, tag="hs")
            ones48 = hp.tile([96, 48], bf16, tag="ones48")
            # whole-tile memsets (partition offsets must be 32-aligned);
            # compute writes then overwrite the non-constant rows
            nc.gpsimd.memset(h1c[2][:, :], 1.0)
            nc.gpsimd.memset(z1s[:, :], 1.0)
            nc.gpsimd.memset(hs[:, :], 1.0)
            nc.gpsimd.memset(ones48[:, :], 1.0)

            # ---- layer 1: h1T[u, c] = relu(W1.T @ xc_b.T + b1), bf16 out
            for i in range(3):
                ps = pp.tile([KC, 96], f32, tag="ps")
                for j in range(3):
                    nc.tensor.matmul(
                        ps, was[:, S * j + KC * i:S * j + KC * (i + 1)],
                        xbs[:, C * j:C * (j + 1)],
                        start=(j == 0), stop=(j == 2))
                nc.scalar.activation(h1c[i][0:KC, :], ps, AF.Relu,
                                     bias=spf_s[:, i:i + 1])

            # ---- layer 2: [tp | sp | tps | sps] with bias ones-row ----
            ps_l2 = pp.tile([96, 194], f32, tag="ps")
            nc.tensor.matmul(ps_l2, h1c[0][0:KC, :], wbs[0:KC, 0:194],
                             start=True, stop=False)
            nc.tensor.matmul(ps_l2, h1c[1][0:KC, :], wbs[0:KC, 194:388],
                             start=False, stop=False)
            nc.tensor.matmul(ps_l2, h1c[2][0:KC + 1, :],
                             wbs[0:KC + 1, 388:582], start=False, stop=True)

            # ts2 (critical) on scalar; at/asl copies off-path on vector
            ts2 = hp.tile([96, 2], bf16, tag="ts2")
            nc.scalar.activation(ts2, ps_l2[:, 192:194], AF.Copy)
            at_s = hp.tile([96, 96], bf16, tag="at_s")
            nc.vector.tensor_copy(at_s, ps_l2[:, 0:96])
            asl_s = hp.tile([96, 96], bf16, tag="asl_s")
            nc.scalar.activation(asl_s, ps_l2[:, 96:192], AF.Copy)

            # ---- fusion softmax: z1 = relu(fn1 @ ts2 + b1f) ----
            ps_z1 = pp.tile([32, 1], f32, tag="ps")
            nc.tensor.matmul(ps_z1, sp16a_s[:, 0:32], ts2[:, 0:1],
                             start=True, stop=False)
            nc.tensor.matmul(ps_z1, sp16a_s[:, 32:64], ts2[:, 1:2],
                             start=False, stop=True)
            nc.scalar.activation(z1s[0:32, :], ps_z1, AF.Relu,
                                 bias=spf_s[0:32, 3:4])

            # z-cols [96c, 3k] directly (no row->col transposes)
            zc = pp.tile([96, 3], f32, tag="ps", name="zc")
            for k in range(3):
                nc.tensor.matmul(zc[:, k:k + 1],
                                 sp16b_s[0:33, 96 + 96 * k:192 + 96 * k],
                                 z1s, skip_group_check=True)
            ec = hp.tile([96, 3], f32, tag="ec")
            rs = hp.tile([96, 1], f32, tag="rs")
            nc.scalar.activation(ec, zc, AF.Exp, accum_out=rs)
            rs_bf = hp.tile([96, 1], bf16, tag="rs_bf")
            nc.scalar.activation(rs_bf, rs, AF.Copy)

            # unnormalized-exp weighted stationaries for the combine
            wt = hp.tile([96, 48], bf16, tag="wt")
            nc.vector.tensor_scalar_mul(wt, sp16a_s[:, 64:112], ec[:, 0:1])
            ws = hp.tile([96, 48], bf16, tag="ws")
            nc.gpsimd.tensor_scalar_mul(ws, sp16a_s[:, 64:112], ec[:, 1:2])
            wd = hp.tile([96, 48], bf16, tag="wd")
            nc.vector.tensor_scalar_mul(wd, sp16a_s[:, 64:112], ec[:, 2:3])

            # ps_h = fp1w @ (e0*tp + e1*sp + e2*dp), unnormalized
            ps_h = pp.tile([48, 96], f32, tag="ps")
            nc.tensor.matmul(ps_h, wt, at_s, start=True, stop=False)
            nc.tensor.matmul(ps_h, ws, asl_s, start=False, stop=False)
            nc.tensor.matmul(ps_h, wd, dpb, start=False, stop=True)

            # denominator -> per-partition recip for the Relu scale
            den48 = pp.tile([48, 1], f32, tag="ps")
            nc.tensor.matmul(den48, ones48, rs_bf, start=True, stop=True)
            recip48 = hp.tile([48, 1], f32, tag="recip48")
            nc.vector.reciprocal(recip48, den48)

            nc.scalar.activation(hs[0:48, :], ps_h, AF.Relu,
                                 bias=spf_s[0:48, 4:5], scale=recip48)

            ps_o = pp.tile([96, 96], f32, tag="ps")
            nc.tensor.matmul(ps_o, hs, sp16b_s[:, 0:96],
                             start=True, stop=True)
            out_s = hp.tile([96, 96], f32, tag="out")
            nc.vector.tensor_copy(out_s, ps_o)
            nc.sync.dma_start(out=y[:, :], in_=out_s)

    nc.compile()
    return nc


def _prep_weights(i):
    f = np.float32
    mm = _mavg_matrix(S, MAIN_K)
    w1 = np.empty((S, 2 * HID), f)
    w1[:, :HID] = mm @ i['lt1w'].T.astype(f)
    w1[:, HID:] = (np.eye(S, dtype=f) - mm) @ i['ls1w'].T.astype(f)
    wa = np.empty((KC, 3 * S), f)
    for j in range(3):
        wa[:, S * j:S * (j + 1)] = w1[KC * j:KC * (j + 1), :]

    # constant detail_pred row (LayerNorm(1) output == ln_b exactly)
    xf = np.full((S,), f(i['ln_b'][0]), f)
    dp_row = (np.maximum(xf @ i['op1w'].T + i['op1b'], 0)
              @ i['op2w'].T + i['op2b']).astype(f)
    dpm = dp_row.mean(dtype=np.float32)
    # z1 bias: only fn1b + dp-mean term (lt2b/ls2b ride the L2 ones-row)
    b1f = (i['fn1b'] + dpm * i['fn1w'][:, 2 * C:].sum(1)).astype(f)

    lt2wt = np.ascontiguousarray(i['lt2w'].T, f)
    ls2wt = np.ascontiguousarray(i['ls2w'].T, f)
    # [337, 194] = [tp 0:96 | sp 96:192 | tps 192 | sps 193]; row 336 = bias
    w2full = np.zeros((S + 1, 194), f)
    w2full[0:HID, 0:96] = lt2wt
    w2full[0:HID, 192] = lt2wt.sum(1)
    w2full[HID:S, 96:192] = ls2wt
    w2full[HID:S, 193] = ls2wt.sum(1)
    w2full[S, 0:96] = i['lt2b']
    w2full[S, 192] = i['lt2b'].sum(dtype=np.float64)
    w2full[S, 96:192] = i['ls2b']
    w2full[S, 193] = i['ls2b'].sum(dtype=np.float64)
    wb = np.zeros((KC + 1, 3 * 194), f)
    for j in range(2):
        wb[0:KC, 194 * j:194 * (j + 1)] = w2full[KC * j:KC * (j + 1)]
    wb[0:KC, 388:582] = w2full[2 * KC:S]
    wb[KC, 388:582] = w2full[S]

    sp16a = np.zeros((96, 112), f)
    sp16a[:, 0:32] = i['fn1w'][:, 0:C].T / C
    sp16a[:, 32:64] = i['fn1w'][:, C:2 * C].T / C
    sp16a[:, 64:112] = i['fp1w'].T

    fn2T = np.ascontiguousarray(i['fn2w'].T, f)
    fn2b = i['fn2b'].astype(f)
    sp16b = np.zeros((49, 480), f)
    sp16b[0:48, 0:96] = i['fp2w'].T
    sp16b[48, 0:96] = i['fp2b']
    for k in range(3):
        sp16b[0:32, 96 + 96 * k:192 + 96 * k] = fn2T[:, 96 * k:96 * (k + 1)]
        sp16b[32, 96 + 96 * k:192 + 96 * k] = fn2b[96 * k:96 * (k + 1)]
    sp16b[0, 384:480] = dp_row

    b1 = np.concatenate([i['lt1b'], i['ls1b']]).astype(f)
    spf = np.zeros((KC, 8), f)
    for j in range(3):
        spf[:, j] = b1[KC * j:KC * (j + 1)]
    spf[0:32, 3] = b1f
    spf[0:48, 4] = i['fp1b']

    return dict(wa=_bf(wa), wb=_bf(wb), sp16a=_bf(sp16a),
                sp16b=_bf(sp16b), spf=spf)


def make_in_maps(inputs):
    shared = _prep_weights(inputs)
    x = np.asarray(inputs['x'], np.float32)
    in_maps = []
    for b in range(N_CORES):
        xbp = np.empty((KC, 3 * C), np.float32)
        for j in range(3):
            xbp[:, C * j:C * (j + 1)] = x[b, KC * j:KC * (j + 1), :]
        in_maps.append(dict(shared, xb=_bf(xbp)))
    return in_maps


def kernel(**inputs):
    if "nc" not in _CACHE:
        _CACHE["nc"] = _build_module()
    res = run_bass_kernel_spmd(_CACHE["nc"], make_in_maps(inputs),
                               core_ids=list(range(N_CORES)))
    return np.stack([res.results[b]["y"] for b in range(N_CORES)], 0)


# revision 6
# speedup vs baseline: 1.0555x; 1.0302x over previous
"""Trainium2 Bass kernel for nn_EnhancedDLinear (8-core SPMD, full I/O).

Mathematical reductions (vs the jax reference, verified numerically):

1. ``LayerNorm(1)`` output is the constant ``ln_b`` (size-1 normalization
   axis), so the detail branch (conv stack, adaptive softmax, [N,S,S]
   attention) is dead code; ``detail_pred`` is a weight-only constant
   row folded on the host.
2. The replicate-pad moving average (k=25) is a linear map folded into
   the first trend/seasonal MLP layers.
3. The channel-mean feeding the fusion MLP folds into its weights; the
   constant detail contribution folds into its bias.
4. Biases ride the matmuls via constant-one contraction rows (no
   broadcast-DMA bias tiles, no vector adds).
5. The fusion softmax normalizer folds into the final hidden Relu's
   per-partition ``scale`` operand, so unnormalized exponentials flow
   through the combine matmuls.

All matmul operands are bf16 (tolerance is 2e-2; measured ~2.6e-3),
halving DMA bytes vs fp32 and running the PE at 1 cycle/row at any
moving width. PSUM accumulation stays fp32.

Sharding: one batch per core (N = B*C, contiguous blocks of C=96), zero
collectives, tiny weights replicated.
"""

import numpy as np
import ml_dtypes

import concourse.bacc as bacc
import concourse.tile as tile
from concourse import mybir
from concourse.bass_utils import run_bass_kernel_spmd

B, S, C, P = 8, 336, 96, 96
HID = 168
MAIN_K = 25
N_CORES = 8
KC = 112          # contraction chunk (336 = 3*112)

_CACHE = {}


def _mavg_matrix(s, k):
    # mt = xc @ Mm for the replicate-padded moving average
    p = (k - 1) // 2
    m = np.zeros((s, s), np.float64)
    for j in range(s):
        for d in range(-p, p + 1):
            i = min(max(j + d, 0), s - 1)
            m[i, j] += 1.0 / k
    return m.astype(np.float32)


def _bf(a):
    return np.ascontiguousarray(a, np.float32).astype(ml_dtypes.bfloat16)


def _build_module():
    f32 = mybir.dt.float32
    bf16 = mybir.dt.bfloat16
    nc = bacc.Bacc("TRN2", target_bir_lowering=False, debug=False,
                   num_devices=N_CORES)

    xb = nc.dram_tensor("xb", [KC, 3 * C], bf16, kind="ExternalInput")
    wa = nc.dram_tensor("wa", [KC, 3 * S], bf16, kind="ExternalInput")
    wb = nc.dram_tensor("wb", [KC + 1, 592], bf16, kind="ExternalInput")
    # sp16a [96, 112]: fn1t (0:32) | fn1s (32:64) | fp1wT (64:112)
    sp16a = nc.dram_tensor("sp16a", [96, 112], bf16, kind="ExternalInput")
    # sp16b [49, 480]: fp2aug rows 0:49 (0:96) | Wk rows 0:33 (96+96k) |
    #                  dp_row row 0 (384:480)
    sp16b = nc.dram_tensor("sp16b", [49, 480], bf16, kind="ExternalInput")
    # spf f32 [112, 8]: b1 u-chunks (cols 0:3) | b1f rows 0:32 (col 3) |
    #                   fp1b rows 0:48 (col 4)
    spf = nc.dram_tensor("spf", [KC, 8], f32, kind="ExternalInput")
    y = nc.dram_tensor("y", [P, P], f32, kind="ExternalOutput")

    AF = mybir.ActivationFunctionType

    with tile.TileContext(nc) as tc:
        with (
            tc.tile_pool(name="wp", bufs=1) as wp,
            tc.tile_pool(name="hp", bufs=1) as hp,
            tc.tile_pool(name="pp", bufs=7, space="PSUM") as pp,
        ):
            xbs = wp.tile([KC, 3 * C], bf16, tag="xbs")
            was = wp.tile([KC, 3 * S], bf16, tag="was")
            wbs = wp.tile([KC + 1, 592], bf16, tag="wbs")
            sp16a_s = wp.tile([96, 112], bf16, tag="sp16a")
            sp16b_s = wp.tile([49, 480], bf16, tag="sp16b")
            spf_s = wp.tile([KC, 8], f32, tag="spf")
            dpb = wp.tile([96, 96], bf16, tag="dpb")

            # DMA issue: wa is the L1 long pole -> its own queue (scalar).
            nc.scalar.dma_start(out=was, in_=wa[:, :])
            nc.sync.dma_start(out=spf_s, in_=spf[:, :])
            nc.sync.dma_start(out=xbs, in_=xb[:, :])
            nc.sync.dma_start(out=wbs, in_=wb[:, :])
            nc.gpsimd.dma_start(out=sp16a_s, in_=sp16a[:, :])
            nc.gpsimd.dma_start(out=sp16b_s, in_=sp16b[:, :])
            nc.gpsimd.dma_start(out=dpb,
                                in_=sp16b[0:1, 384:480].broadcast_to((96, 96)))

            # constant-one rows / tiles (off the DMA critical path)
            h1c = [hp.tile([KC + 1, 96], bf16, tag=f"h1c_{j}",
                           name=f"h1c_{j}") for j in range(3)]
            z1s = hp.tile([33, 1], bf16, tag="z1s")
            hs = hp.tile([49, 96], bf16, tag="hs")
            ones48 = hp.tile([96, 48], bf16, tag="ones48")
            # whole-tile memsets (partition offsets must be 32-aligned);
            # compute writes then overwrite the non-constant rows
            nc.gpsimd.memset(h1c[2][:, :], 1.0)
            nc.gpsimd.memset(z1s[:, :], 1.0)
            nc.gpsimd.memset(hs[:, :], 1.0)
            nc.gpsimd.memset(ones48[:, :], 1.0)

            # ---- layer 1: h1T[u, c] = relu(W1.T @ xc_b.T + b1), bf16 out
            for i in range(3):
                ps = pp.tile([KC, 96], f32, tag="ps")
                for j in range(3):
                    nc.tensor.matmul(
                        ps, was[:, S * j + KC * i:S * j + KC * (i + 1)],
                        xbs[:, C * j:C * (j + 1)],
                        start=(j == 0), stop=(j == 2))
                nc.scalar.activation(h1c[i][0:KC, :], ps, AF.Relu,
                                     bias=spf_s[:, i:i + 1])

            # ---- layer 2: [tp | sp | tps | sps] with bias ones-row ----
            ps_l2 = pp.tile([96, 194], f32, tag="ps")
            nc.tensor.matmul(ps_l2, h1c[0][0:KC, :], wbs[0:KC, 0:194],
                             start=True, stop=False)
            nc.tensor.matmul(ps_l2, h1c[1][0:KC, :], wbs[0:KC, 194:388],
                             start=False, stop=False)
            nc.tensor.matmul(ps_l2, h1c[2][0:KC + 1, :],
                             wbs[0:KC + 1, 388:582], start=False, stop=True)

            # ts2 (critical) on scalar; at/asl copies off-path on vector
            ts2 = hp.tile([96, 2], bf16, tag="ts2")
            nc.scalar.activation(ts2, ps_l2[:, 192:194], AF.Copy)
            at_s = hp.tile([96, 96], bf16, tag="at_s")
            nc.vector.tensor_copy(at_s, ps_l2[:, 0:96])
            asl_s = hp.tile([96, 96], bf16, tag="asl_s")
            nc.vector.tensor_copy(asl_s, ps_l2[:, 96:192])

            # ---- fusion softmax: z1 = relu(fn1 @ ts2 + b1f) ----
            ps_z1 = pp.tile([32, 1], f32, tag="ps")
            nc.tensor.matmul(ps_z1, sp16a_s[:, 0:32], ts2[:, 0:1],
                             start=True, stop=False)
            nc.tensor.matmul(ps_z1, sp16a_s[:, 32:64], ts2[:, 1:2],
                             start=False, stop=True)
            nc.scalar.activation(z1s[0:32, :], ps_z1, AF.Relu,
                                 bias=spf_s[0:32, 3:4])

            # z-cols [96c, 3k] directly (no row->col transposes)
            zc = pp.tile([96, 3], f32, tag="ps", name="zc")
            for k in range(3):
                nc.tensor.matmul(zc[:, k:k + 1],
                                 sp16b_s[0:33, 96 + 96 * k:192 + 96 * k],
                                 z1s, skip_group_check=True)
            ec = hp.tile([96, 3], f32, tag="ec")
            rs = hp.tile([96, 1], f32, tag="rs")
            nc.scalar.activation(ec, zc, AF.Exp, accum_out=rs)
            rs_bf = hp.tile([96, 1], bf16, tag="rs_bf")
            nc.scalar.activation(rs_bf, rs, AF.Copy)

            # unnormalized-exp weighted stationaries for the combine
            wt = hp.tile([96, 48], bf16, tag="wt")
            nc.vector.tensor_scalar_mul(wt, sp16a_s[:, 64:112], ec[:, 0:1])
            ws = hp.tile([96, 48], bf16, tag="ws")
            nc.scalar.activation(ws, sp16a_s[:, 64:112], AF.Copy,
                                 scale=ec[:, 1:2])
            wd = hp.tile([96, 48], bf16, tag="wd")
            nc.vector.tensor_scalar_mul(wd, sp16a_s[:, 64:112], ec[:, 2:3])

            # denominator -> per-partition recip for the Relu scale
            den48 = pp.tile([48, 1], f32, tag="ps")
            nc.tensor.matmul(den48, ones48, rs_bf, start=True, stop=True)
            recip48 = hp.tile([48, 1], f32, tag="recip48")
            nc.vector.reciprocal(recip48, den48)

            # ps_h = fp1w @ (e0*tp + e1*sp + e2*dp), unnormalized
            ps_h = pp.tile([48, 96], f32, tag="ps")
            nc.tensor.matmul(ps_h, wt, at_s, start=True, stop=False)
            nc.tensor.matmul(ps_h, ws, asl_s, start=False, stop=False)
            nc.tensor.matmul(ps_h, wd, dpb, start=False, stop=True)

            nc.scalar.activation(hs[0:48, :], ps_h, AF.Relu,
                                 bias=spf_s[0:48, 4:5], scale=recip48)

            ps_o = pp.tile([96, 96], f32, tag="ps")
            nc.tensor.matmul(ps_o, hs, sp16b_s[:, 0:96],
                             start=True, stop=True)
            out_s = hp.tile([96, 96], f32, tag="out")
            nc.vector.tensor_copy(out_s, ps_o)
            nc.sync.dma_start(out=y[:, :], in_=out_s)

    nc.compile()
    return nc


def _prep_weights(i):
    f = np.float32
    mm = _mavg_matrix(S, MAIN_K)
    w1 = np.empty((S, 2 * HID), f)
    w1[:, :HID] = mm @ i['lt1w'].T.astype(f)
    w1[:, HID:] = (np.eye(S, dtype=f) - mm) @ i['ls1w'].T.astype(f)
    wa = np.empty((KC, 3 * S), f)
    for j in range(3):
        wa[:, S * j:S * (j + 1)] = w1[KC * j:KC * (j + 1), :]

    # constant detail_pred row (LayerNorm(1) output == ln_b exactly)
    xf = np.full((S,), f(i['ln_b'][0]), f)
    dp_row = (np.maximum(xf @ i['op1w'].T + i['op1b'], 0)
              @ i['op2w'].T + i['op2b']).astype(f)
    dpm = dp_row.mean(dtype=np.float32)
    # z1 bias: only fn1b + dp-mean term (lt2b/ls2b ride the L2 ones-row)
    b1f = (i['fn1b'] + dpm * i['fn1w'][:, 2 * C:].sum(1)).astype(f)

    lt2wt = np.ascontiguousarray(i['lt2w'].T, f)
    ls2wt = np.ascontiguousarray(i['ls2w'].T, f)
    # [337, 194] = [tp 0:96 | sp 96:192 | tps 192 | sps 193]; row 336 = bias
    w2full = np.zeros((S + 1, 194), f)
    w2full[0:HID, 0:96] = lt2wt
    w2full[0:HID, 192] = lt2wt.sum(1)
    w2full[HID:S, 96:192] = ls2wt
    w2full[HID:S, 193] = ls2wt.sum(1)
    w2full[S, 0:96] = i['lt2b']
    w2full[S, 192] = i['lt2b'].sum(dtype=np.float64)
    w2full[S, 96:192] = i['ls2b']
    w2full[S, 193] = i['ls2b'].sum(dtype=np.float64)
    wb = np.zeros((KC + 1, 592), f)
    for j in range(2):
        wb[0:KC, 194 * j:194 * (j + 1)] = w2full[KC * j:KC * (j + 1)]
    wb[0:KC, 388:582] = w2full[2 * KC:S]
    wb[KC, 388:582] = w2full[S]

    sp16a = np.zeros((96, 112), f)
    sp16a[:, 0:32] = i['fn1w'][:, 0:C].T / C
    sp16a[:, 32:64] = i['fn1w'][:, C:2 * C].T / C
    sp16a[:, 64:112] = i['fp1w'].T

    fn2T = np.ascontiguousarray(i['fn2w'].T, f)
    fn2b = i['fn2b'].astype(f)
    sp16b = np.zeros((49, 480), f)
    sp16b[0:48, 0:96] = i['fp2w'].T
    sp16b[48, 0:96] = i['fp2b']
    for k in range(3):
        sp16b[0:32, 96 + 96 * k:192 + 96 * k] = fn2T[:, 96 * k:96 * (k + 1)]
        sp16b[32, 96 + 96 * k:192 + 96 * k] = fn2b[96 * k:96 * (k + 1)]
    sp16b[0, 384:480] = dp_row

    b1 = np.concatenate([i['lt1b'], i['ls1b']]).astype(f)
    spf = np.zeros((KC, 8), f)
    for j in range(3):
        spf[:, j] = b1[KC * j:KC * (j + 1)]
    spf[0:32, 3] = b1f
    spf[0:48, 4] = i['fp1b']

    return dict(wa=_bf(wa), wb=_bf(wb), sp16a=_bf(sp16a),
                sp16b=_bf(sp16b), spf=spf)


def make_in_maps(inputs):
    shared = _prep_weights(inputs)
    x = np.asarray(inputs['x'], np.float32)
    in_maps = []
    for b in range(N_CORES):
        xbp = np.empty((KC, 3 * C), np.float32)
        for j in range(3):
            xbp[:, C * j:C * (j + 1)] = x[b, KC * j:KC * (j + 1), :]
        in_maps.append(dict(shared, xb=_bf(xbp)))
    return in_maps


def kernel(**inputs):
    if "nc" not in _CACHE:
        _CACHE["nc"] = _build_module()
    res = run_bass_kernel_spmd(_CACHE["nc"], make_in_maps(inputs),
                               core_ids=list(range(N_CORES)))
    return np.stack([res.results[b]["y"] for b in range(N_CORES)], 0)


# revision 7
# speedup vs baseline: 1.2979x; 1.2297x over previous
"""Trainium2 Bass kernel for nn_EnhancedDLinear (8-core SPMD, full I/O).

Mathematical reductions (vs the jax reference, verified numerically):

1. ``LayerNorm(1)`` output is the constant ``ln_b`` (size-1 normalization
   axis), so the detail branch (conv stack, adaptive softmax, [N,S,S]
   attention) is dead code; ``detail_pred`` is a weight-only constant
   row folded on the host.
2. The replicate-pad moving average (k=25) is a linear map folded into
   the first trend/seasonal MLP layers.
3. The channel-mean feeding the fusion MLP folds into its weights; the
   constant detail contribution folds into its bias.
4. Biases ride the matmuls via constant-one contraction rows (no
   broadcast-DMA bias tiles, no vector adds).
5. The fusion softmax normalizer folds into the final hidden Relu's
   per-partition ``scale`` operand, so unnormalized exponentials flow
   through the combine matmuls.

All matmul operands are bf16 (tolerance is 2e-2; measured ~2.6e-3),
halving DMA bytes vs fp32 and running the PE at 1 cycle/row at any
moving width. PSUM accumulation stays fp32.

Sharding: one batch per core (N = B*C, contiguous blocks of C=96), zero
collectives, tiny weights replicated.
"""

import numpy as np
import ml_dtypes

import concourse.bacc as bacc
import concourse.tile as tile
from concourse import mybir
from concourse.bass_utils import run_bass_kernel_spmd

B, S, C, P = 8, 336, 96, 96
HID = 168
MAIN_K = 25
N_CORES = 8
KC = 112          # contraction chunk (336 = 3*112)

_CACHE = {}


def _mavg_matrix(s, k):
    # mt = xc @ Mm for the replicate-padded moving average
    p = (k - 1) // 2
    m = np.zeros((s, s), np.float64)
    for j in range(s):
        for d in range(-p, p + 1):
            i = min(max(j + d, 0), s - 1)
            m[i, j] += 1.0 / k
    return m.astype(np.float32)


def _bf(a):
    return np.ascontiguousarray(a, np.float32).astype(ml_dtypes.bfloat16)


def _build_module():
    f32 = mybir.dt.float32
    bf16 = mybir.dt.bfloat16
    nc = bacc.Bacc("TRN2", target_bir_lowering=False, debug=False,
                   num_devices=N_CORES)

    xb = nc.dram_tensor("xb", [KC, 3 * C], bf16, kind="ExternalInput")
    wa = nc.dram_tensor("wa", [KC, 3 * S], bf16, kind="ExternalInput")
    wb = nc.dram_tensor("wb", [128, 592], bf16, kind="ExternalInput")
    # sp16a [96, 112]: fn1t (0:32) | fn1s (32:64) | fp1wT (64:112)
    sp16a = nc.dram_tensor("sp16a", [96, 112], bf16, kind="ExternalInput")
    # sp16b [49, 480]: fp2aug rows 0:49 (0:96) | Wk rows 0:33 (96+96k) |
    #                  dp_row row 0 (384:480)
    sp16b = nc.dram_tensor("sp16b", [49, 480], bf16, kind="ExternalInput")
    # spf f32 [112, 8]: b1 u-chunks (cols 0:3) | b1f rows 0:32 (col 3) |
    #                   fp1b rows 0:48 (col 4)
    spf = nc.dram_tensor("spf", [KC, 8], f32, kind="ExternalInput")
    y = nc.dram_tensor("y", [P, P], f32, kind="ExternalOutput")

    AF = mybir.ActivationFunctionType

    with tile.TileContext(nc) as tc:
        with (
            tc.tile_pool(name="wp", bufs=1) as wp,
            tc.tile_pool(name="hp", bufs=1) as hp,
            tc.tile_pool(name="pp", bufs=7, space="PSUM") as pp,
        ):
            xbs = wp.tile([KC, 3 * C], bf16, tag="xbs")
            was = wp.tile([KC, 3 * S], bf16, tag="was")
            wbs = wp.tile([128, 592], bf16, tag="wbs")
            sp16a_s = wp.tile([96, 112], bf16, tag="sp16a")
            sp16b_s = wp.tile([49, 480], bf16, tag="sp16b")
            spf_s = wp.tile([KC, 8], f32, tag="spf")
            dpb = wp.tile([96, 96], bf16, tag="dpb")

            # DMA issue: wa is the L1 long pole -> its own queue (scalar).
            nc.scalar.dma_start(out=was, in_=wa[:, :])
            nc.sync.dma_start(out=spf_s, in_=spf[:, :])
            nc.sync.dma_start(out=xbs, in_=xb[:, :])
            nc.sync.dma_start(out=wbs, in_=wb[:, :])
            nc.gpsimd.dma_start(out=sp16a_s, in_=sp16a[:, :])
            nc.gpsimd.dma_start(out=sp16b_s, in_=sp16b[:, :])
            nc.gpsimd.dma_start(out=dpb,
                                in_=sp16b[0:1, 384:480].broadcast_to((96, 96)))

            # constant-one rows / tiles (off the DMA critical path)
            h1c = [hp.tile([KC + 1, 96], bf16, tag=f"h1c_{j}",
                           name=f"h1c_{j}") for j in range(3)]
            z1s = hp.tile([33, 1], bf16, tag="z1s")
            hs = hp.tile([49, 96], bf16, tag="hs")
            ones48 = hp.tile([96, 48], f32, tag="ones48")
            # whole-tile memsets (partition offsets must be 32-aligned);
            # compute writes then overwrite the non-constant rows
            nc.gpsimd.memset(h1c[2][:, :], 1.0)
            nc.gpsimd.memset(z1s[:, :], 1.0)
            nc.gpsimd.memset(hs[:, :], 1.0)
            nc.gpsimd.memset(ones48[:, :], 1.0)

            # ---- layer 1: h1T[u, c] = relu(W1.T @ xc_b.T + b1), bf16 out
            for i in range(3):
                ps = pp.tile([KC, 96], f32, tag="ps")
                for j in range(3):
                    nc.tensor.matmul(
                        ps, was[:, S * j + KC * i:S * j + KC * (i + 1)],
                        xbs[:, C * j:C * (j + 1)],
                        start=(j == 0), stop=(j == 2))
                nc.scalar.activation(h1c[i][0:KC, :], ps, AF.Relu,
                                     bias=spf_s[:, i:i + 1])

            # ---- layer 2: [tp | sp | tps | sps] with bias ones-row ----
            ps_l2 = pp.tile([96, 194], f32, tag="ps")
            nc.tensor.matmul(ps_l2, h1c[0][0:KC, :], wbs[0:KC, 0:194],
                             start=True, stop=False)
            nc.tensor.matmul(ps_l2, h1c[1][0:KC, :], wbs[0:KC, 194:388],
                             start=False, stop=False)
            nc.tensor.matmul(ps_l2, h1c[2][0:KC + 1, :],
                             wbs[0:KC + 1, 388:582], start=False, stop=True)

            # ts2 (critical) on scalar; at/asl copies off-path on vector
            ts2 = hp.tile([96, 2], bf16, tag="ts2")
            nc.scalar.activation(ts2, ps_l2[:, 192:194], AF.Copy)
            at_s = hp.tile([96, 96], bf16, tag="at_s")
            nc.vector.tensor_copy(at_s, ps_l2[:, 0:96])
            asl_s = hp.tile([96, 96], bf16, tag="asl_s")
            nc.vector.tensor_copy(asl_s, ps_l2[:, 96:192])

            # ---- fusion softmax: z1 = relu(fn1 @ ts2 + b1f) ----
            ps_z1 = pp.tile([32, 1], f32, tag="ps")
            nc.tensor.matmul(ps_z1, sp16a_s[:, 0:32], ts2[:, 0:1],
                             start=True, stop=False)
            nc.tensor.matmul(ps_z1, sp16a_s[:, 32:64], ts2[:, 1:2],
                             start=False, stop=True)
            nc.scalar.activation(z1s[0:32, :], ps_z1, AF.Relu,
                                 bias=spf_s[0:32, 3:4])

            # z-cols [96c, 3k] directly (no row->col transposes)
            zc = pp.tile([96, 3], f32, tag="ps", name="zc")
            for k in range(3):
                nc.tensor.matmul(zc[:, k:k + 1],
                                 sp16b_s[0:33, 96 + 96 * k:192 + 96 * k],
                                 z1s, skip_group_check=True)
            ec = hp.tile([96, 3], f32, tag="ec")
            rs = hp.tile([96, 1], f32, tag="rs")
            nc.scalar.activation(ec, zc, AF.Exp, accum_out=rs)

            # unnormalized-exp weighted stationaries for the combine
            wt = hp.tile([96, 48], bf16, tag="wt")
            nc.vector.tensor_scalar_mul(wt, sp16a_s[:, 64:112], ec[:, 0:1])
            ws = hp.tile([96, 48], bf16, tag="ws")
            nc.vector.tensor_scalar_mul(ws, sp16a_s[:, 64:112], ec[:, 1:2])
            wd = hp.tile([96, 48], bf16, tag="wd")
            nc.vector.tensor_scalar_mul(wd, sp16a_s[:, 64:112], ec[:, 2:3])

            # denominator -> per-partition recip for the Relu scale
            den48 = pp.tile([48, 1], f32, tag="ps")
            nc.tensor.matmul(den48, ones48, rs, start=True, stop=True)
            recip48 = hp.tile([48, 1], f32, tag="recip48")
            nc.vector.reciprocal(recip48, den48)

            # ps_h = fp1w @ (e0*tp + e1*sp + e2*dp), unnormalized
            ps_h = pp.tile([48, 96], f32, tag="ps")
            nc.tensor.matmul(ps_h, wt, at_s, start=True, stop=False)
            nc.tensor.matmul(ps_h, ws, asl_s, start=False, stop=False)
            nc.tensor.matmul(ps_h, wd, dpb, start=False, stop=True)

            nc.scalar.activation(hs[0:48, :], ps_h, AF.Relu,
                                 bias=spf_s[0:48, 4:5], scale=recip48)

            ps_o = pp.tile([96, 96], f32, tag="ps")
            nc.tensor.matmul(ps_o, hs, sp16b_s[:, 0:96],
                             start=True, stop=True)
            out_s = hp.tile([96, 96], f32, tag="out")
            nc.vector.tensor_copy(out_s, ps_o)
            nc.sync.dma_start(out=y[:, :], in_=out_s)

    nc.compile()
    return nc


def _prep_weights(i):
    f = np.float32
    mm = _mavg_matrix(S, MAIN_K)
    w1 = np.empty((S, 2 * HID), f)
    w1[:, :HID] = mm @ i['lt1w'].T.astype(f)
    w1[:, HID:] = (np.eye(S, dtype=f) - mm) @ i['ls1w'].T.astype(f)
    wa = np.empty((KC, 3 * S), f)
    for j in range(3):
        wa[:, S * j:S * (j + 1)] = w1[KC * j:KC * (j + 1), :]

    # constant detail_pred row (LayerNorm(1) output == ln_b exactly)
    xf = np.full((S,), f(i['ln_b'][0]), f)
    dp_row = (np.maximum(xf @ i['op1w'].T + i['op1b'], 0)
              @ i['op2w'].T + i['op2b']).astype(f)
    dpm = dp_row.mean(dtype=np.float32)
    # z1 bias: only fn1b + dp-mean term (lt2b/ls2b ride the L2 ones-row)
    b1f = (i['fn1b'] + dpm * i['fn1w'][:, 2 * C:].sum(1)).astype(f)

    lt2wt = np.ascontiguousarray(i['lt2w'].T, f)
    ls2wt = np.ascontiguousarray(i['ls2w'].T, f)
    # [337, 194] = [tp 0:96 | sp 96:192 | tps 192 | sps 193]; row 336 = bias
    w2full = np.zeros((S + 1, 194), f)
    w2full[0:HID, 0:96] = lt2wt
    w2full[0:HID, 192] = lt2wt.sum(1)
    w2full[HID:S, 96:192] = ls2wt
    w2full[HID:S, 193] = ls2wt.sum(1)
    w2full[S, 0:96] = i['lt2b']
    w2full[S, 192] = i['lt2b'].sum(dtype=np.float64)
    w2full[S, 96:192] = i['ls2b']
    w2full[S, 193] = i['ls2b'].sum(dtype=np.float64)
    wb = np.zeros((128, 592), f)
    for j in range(2):
        wb[0:KC, 194 * j:194 * (j + 1)] = w2full[KC * j:KC * (j + 1)]
    wb[0:KC, 388:582] = w2full[2 * KC:S]
    wb[KC, 388:582] = w2full[S]

    sp16a = np.zeros((96, 112), f)
    sp16a[:, 0:32] = i['fn1w'][:, 0:C].T / C
    sp16a[:, 32:64] = i['fn1w'][:, C:2 * C].T / C
    sp16a[:, 64:112] = i['fp1w'].T

    fn2T = np.ascontiguousarray(i['fn2w'].T, f)
    fn2b = i['fn2b'].astype(f)
    sp16b = np.zeros((49, 480), f)
    sp16b[0:48, 0:96] = i['fp2w'].T
    sp16b[48, 0:96] = i['fp2b']
    for k in range(3):
        sp16b[0:32, 96 + 96 * k:192 + 96 * k] = fn2T[:, 96 * k:96 * (k + 1)]
        sp16b[32, 96 + 96 * k:192 + 96 * k] = fn2b[96 * k:96 * (k + 1)]
    sp16b[0, 384:480] = dp_row

    b1 = np.concatenate([i['lt1b'], i['ls1b']]).astype(f)
    spf = np.zeros((KC, 8), f)
    for j in range(3):
        spf[:, j] = b1[KC * j:KC * (j + 1)]
    spf[0:32, 3] = b1f
    spf[0:48, 4] = i['fp1b']

    return dict(wa=_bf(wa), wb=_bf(wb), sp16a=_bf(sp16a),
                sp16b=_bf(sp16b), spf=spf)


def make_in_maps(inputs):
    shared = _prep_weights(inputs)
    x = np.asarray(inputs['x'], np.float32)
    in_maps = []
    for b in range(N_CORES):
        xbp = np.empty((KC, 3 * C), np.float32)
        for j in range(3):
            xbp[:, C * j:C * (j + 1)] = x[b, KC * j:KC * (j + 1), :]
        in_maps.append(dict(shared, xb=_bf(xbp)))
    return in_maps


def kernel(**inputs):
    if "nc" not in _CACHE:
        _CACHE["nc"] = _build_module()
    res = run_bass_kernel_spmd(_CACHE["nc"], make_in_maps(inputs),
                               core_ids=list(range(N_CORES)))
    return np.stack([res.results[b]["y"] for b in range(N_CORES)], 0)
